# revision 1
# baseline (speedup 1.0000x reference)
"""Trainium2 Bass kernel for nn_Decoder_LSTM: 12-step LSTM over (16, 10000, 64).

Sharding: rows = B*N = 160000 flattened, 20000 rows per core (data-parallel,
2 batches/core); gate + edge weights replicated on all 8 cores.

Per-core layout (feature-major, two 10000-row halves packed into 128
partitions):
  XHa (128, 10000) f32r : partitions 0:64 = x^T (half A), 64:128 = h^T (half A)
  XHb (128, 10000) f32r : partitions 0:64 = h^T (half B), 64:128 = x^T (half B)
  C2  (128, 10000) f32  : partitions 0:64 = c (half A), 64:128 = c (half B)

Per step, per 512-column chunk: 8 col-tiled float32r matmuls produce the four
gate pre-activations dual-packed in PSUM; ScalarE applies sigmoid/tanh with
per-partition bias; VectorE does the cell update; 2 matmuls + sigmoid give
y^T which is DMA'd out feature-major (12, 64, 20000). The host reassembles
the (12, 16, 10000, 64) output.
"""
import numpy as np

T, B, N, F = 12, 16, 10000, 64
R_TOTAL = B * N
N_CORES = 8
R = R_TOTAL // N_CORES   # 20000 rows per core
RH = R // 2              # 10000 per half
FD = 1000          # rows per chunk (two 500-wide PSUM-bank regions)
REG = 500          # region width within a chunk
SLOT = 1024        # psum tile allocation width (2 banks)
CHUNKS = [(i * FD, FD) for i in range(RH // FD)]

_NC = None
LAST_EXEC_NS = None
MM_DT = "f32r"   # "f32r" | "bf16" for matmul operand dtype


def _build():
    from contextlib import ExitStack
    from concourse import bacc, mybir
    import concourse.tile as tile

    f32 = mybir.dt.float32
    f32r = mybir.dt.float32r if MM_DT == "f32r" else mybir.dt.bfloat16
    AF = mybir.ActivationFunctionType

    nc = bacc.Bacc(trn_type="TRN2")
    x_in = nc.dram_tensor("xT", [F, R], f32, kind="ExternalInput")
    gw_in = nc.dram_tensor("gw", [128, 1024], f32, kind="ExternalInput")
    we_in = nc.dram_tensor("we", [128, 256], f32, kind="ExternalInput")
    bias_in = nc.dram_tensor("bias", [128, 4], f32, kind="ExternalInput")
    out = nc.dram_tensor("out", [T, F, R], f32, kind="ExternalOutput")

    # gate ACT functions in (i, f, g, o) order
    GATE_FUNC = [AF.Sigmoid, AF.Sigmoid, AF.Tanh, AF.Sigmoid]

    with tile.TileContext(nc) as tc, ExitStack() as ctx:
        fixed = ctx.enter_context(tc.tile_pool(name="fixed", bufs=1))
        state = ctx.enter_context(tc.tile_pool(name="state", bufs=1))
        work = ctx.enter_context(tc.tile_pool(name="work", bufs=2))
        ypool = ctx.enter_context(tc.tile_pool(name="ypool", bufs=2))
        psum = ctx.enter_context(tc.tile_pool(name="psum", bufs=1, space="PSUM"))

        def gv(ap):
            """gapped 3-D view of a (128, SLOT) psum tile: [p, 2, REG]."""
            return ap.rearrange("p (b f) -> p b f", b=2)[:, :, 0:REG]

        # ---- fixed tensors -------------------------------------------------
        gw_f = fixed.tile([128, 1024], f32)
        nc.sync.dma_start(gw_f[:], gw_in[:])
        W = fixed.tile([128, 1024], f32r)
        nc.vector.tensor_copy(W[:], gw_f[:])

        we_f = fixed.tile([128, 256], f32)
        nc.sync.dma_start(we_f[:], we_in[:])
        WE = fixed.tile([128, 256], f32r)
        nc.vector.tensor_copy(WE[:], we_f[:])

        bias_t = fixed.tile([128, 4], f32)
        nc.sync.dma_start(bias_t[:], bias_in[:])

        # ---- persistent state (per-chunk tiles) ----------------------------
        NCH = len(CHUNKS)
        XHa = [state.tile([128, FD], f32r, tag=f"xha{j}", name=f"xha{j}") for j in range(NCH)]
        XHb = [state.tile([128, FD], f32r, tag=f"xhb{j}", name=f"xhb{j}") for j in range(NCH)]
        C2 = [state.tile([128, FD], f32, tag=f"c2{j}", name=f"c2{j}") for j in range(NCH)]
        for j in range(NCH):
            nc.vector.memset(C2[j][:], 0.0)
            nc.vector.tensor_copy(XHa[j][64:128, :], C2[j][0:64, :])
            nc.vector.tensor_copy(XHb[j][0:64, :], C2[j][0:64, :])

        # ---- input load: x arrives pre-transposed (64, R) ------------------
        # DMA into a staging tile, then one rounding copy into the f32r XH
        # x-half (f32r operands must be produced by a compute op)
        for half, (roff, xhl, pbase) in enumerate([(0, XHa, 0), (RH, XHb, 64)]):
            for j, (c0, cw) in enumerate(CHUNKS):
                xr = work.tile([64, FD], f32, tag="xr")
                nc.sync.dma_start(xr[:], x_in[:, roff + c0:roff + c0 + cw])
                nc.vector.tensor_copy(xhl[j][pbase:pbase + 64, :], xr[:])

        # ---- time loop (1-chunk software pipeline: gates(j) then tail(j-1),
        # so ACT's tanh/sigmoid-y of a chunk never stalls on the DVE cell
        # update of the same chunk) -----------------------------------------
        pending = None   # (t, j, gates_s)

        def emit_gates(t, j):
            gates_s = []
            for q in range(4):
                ps_q = psum.tile([128, SLOT], f32, tag=f"ps{q % 3}")
                for r in range(2):
                    rr = slice(r * REG, (r + 1) * REG)
                    pr = ps_q[:, r * 512:r * 512 + REG]
                    nc.tensor.matmul(
                        pr, W[:, q * 256:q * 256 + 128],
                        XHa[j][:, rr], start=True, stop=False,
                    )
                    nc.tensor.matmul(
                        pr, W[:, q * 256 + 128:(q + 1) * 256],
                        XHb[j][:, rr], start=False, stop=True,
                    )
                s_q = work.tile([128, FD], f32, tag=f"s{q}", bufs=3)
                nc.scalar.activation(
                    s_q[:], gv(ps_q[:]), GATE_FUNC[q],
                    bias=bias_t[:, q:q + 1],
                )
                gates_s.append(s_q)
            return gates_s

        def emit_tail(t, j, gates_s):
            c0, cw = CHUNKS[j]
            si, sf, tg, so = gates_s
            m1 = work.tile([128, FD], f32, tag="m1", bufs=1)
            nc.vector.tensor_mul(m1[:], si[:], tg[:])
            m2 = work.tile([128, FD], f32, tag="m2", bufs=1)
            nc.vector.tensor_mul(m2[:], sf[:], C2[j][:])
            nc.vector.tensor_add(C2[j][:], m1[:], m2[:])
            tc_t = work.tile([128, FD], f32, tag="tc")
            nc.scalar.activation(tc_t[:], C2[j][:], AF.Tanh)
            nc.vector.tensor_mul(XHa[j][64:128, :], so[0:64, :], tc_t[0:64, :])
            nc.vector.tensor_mul(XHb[j][0:64, :], so[64:128, :], tc_t[64:128, :])
            yo = ypool.tile([128, FD], f32, tag="yo")
            for r in range(2):
                rr = slice(r * REG, (r + 1) * REG)
                ps_y = psum.tile([128, 512], f32, tag="psy", bufs=2)
                nc.tensor.matmul(
                    ps_y[:, 0:REG], WE[64:128, 0:128], XHa[j][64:128, rr],
                    start=True, stop=False,
                )
                nc.tensor.matmul(
                    ps_y[:, 0:REG], WE[0:64, 128:256], XHb[j][0:64, rr],
                    start=False, stop=True,
                )
                nc.scalar.activation(yo[:, rr], ps_y[:, 0:REG], AF.Sigmoid)
            nc.sync.dma_start(out[t, :, c0:c0 + cw], yo[0:64, :])
            nc.sync.dma_start(out[t, :, RH + c0:RH + c0 + cw], yo[64:128, :])

        for t in range(T):
            for j in range(len(CHUNKS)):
                gates_s = emit_gates(t, j)
                if pending is not None:
                    emit_tail(*pending)
                pending = (t, j, gates_s)
        emit_tail(*pending)

    nc.finalize()
    return nc


def _prep_shared(gate_w, gate_b, W_edge):
    """Host-side packing of the replicated weight tensors."""
    gw = np.asarray(gate_w, dtype=np.float32)          # (256, 128) = (4F, 2F)
    gb = np.asarray(gate_b, dtype=np.float32)          # (256,)
    we = np.asarray(W_edge, dtype=np.float32)          # (64, 64)

    # lhsT for half A: XHa rows = [x(64); h(64)] -> columns of gate_w as-is
    # lhsT for half B: XHb rows = [h(64); x(64)] -> swap the x/h column blocks
    gwT = gw.T                                          # (128, 256): [x;h] rows, gate cols
    gwT_swap = np.concatenate([gwT[64:128], gwT[0:64]], axis=0)
    # per gate q: [A-block (cols 0:64 = weights, 64:128 = 0) | B-block (cols
    # 0:64 = 0, 64:128 = swapped weights)], each (128, 128)
    gw_pack = np.zeros((128, 1024), dtype=np.float32)
    for q in range(4):
        gw_pack[:, q * 256:q * 256 + 64] = gwT[:, q * 64:(q + 1) * 64]
        gw_pack[:, q * 256 + 192:(q + 1) * 256] = gwT_swap[:, q * 64:(q + 1) * 64]

    we_pack = np.zeros((128, 256), dtype=np.float32)
    we_pack[64:128, 0:64] = we         # y_a lhsT: h_a at partitions 64:128 -> y partitions 0:64
    we_pack[0:64, 192:256] = we        # y_b lhsT: h_b at partitions 0:64 -> y partitions 64:128

    bias_pack = np.zeros((128, 4), dtype=np.float32)
    for q in range(4):
        bq = gb[q * 64:(q + 1) * 64]
        bias_pack[0:64, q] = bq
        bias_pack[64:128, q] = bq

    ident = np.eye(128, dtype=np.float32)
    return gw_pack, we_pack, bias_pack, ident


def kernel(inputs_edge, gate_w, gate_b, W_edge):
    from concourse.bass_utils import run_bass_kernel_spmd

    global _NC
    if _NC is None:
        _NC = _build()

    x_T = np.ascontiguousarray(
        np.asarray(inputs_edge, dtype=np.float32).reshape(R_TOTAL, F).T
    )  # (64, R_TOTAL)
    gw_pack, we_pack, bias_pack, _ = _prep_shared(gate_w, gate_b, W_edge)

    in_maps = []
    for c in range(N_CORES):
        in_maps.append({
            "xT": np.ascontiguousarray(x_T[:, c * R:(c + 1) * R]),
            "gw": gw_pack,
            "we": we_pack,
            "bias": bias_pack,
        })

    import os
    global LAST_EXEC_NS
    trace = bool(os.environ.get("KTRACE"))
    res = run_bass_kernel_spmd(
        _NC, in_maps, core_ids=list(range(N_CORES)), trace=trace,
        trace_cores=[0] if trace else None,
    )
    if res.exec_time_ns is not None:
        LAST_EXEC_NS = res.exec_time_ns
    # per-core (T, F, R) feature-major -> full (T, B, N, F)
    full = np.concatenate([r["out"] for r in res.results], axis=2)  # (T, F, R_TOTAL)
    return np.ascontiguousarray(full.transpose(0, 2, 1)).reshape(T, B, N, F)

